# revision 1
# baseline (speedup 1.0000x reference)
"""Trainium2 Bass kernel for a nonstandard GRU (gates computed after state update).

Strategy: data-parallel over batch (64 samples -> 8 cores x 8 samples).
Per core, the T=512 sequential recurrence runs entirely from SBUF:
  - gate matmuls stream weights through 4 concurrent PE column-groups
    (stationary = h^T tiles [128,8], moving = W^T chunks [128,256])
  - gate outputs land "striped" in PSUM: chunk g at partitions [32g, 32g+8),
    so elementwise/activation ops see FD=256 on 104 partitions instead of
    FD=1024 on 8 partitions.
  - input projections (X @ Wx^T etc.) are folded into the recurrent matmul
    as 2 extra K-tiles (lhsT = x_t^T staged [128,16]); bias as a K=1 tile.
  - h' and h'*r are transposed back to lhsT layout via PE transpose.
"""

import os
import sys

sys.path.insert(0, "/opt/trn_rl_repo")

import numpy as np

import concourse.bass as bass
import concourse.mybir as mybir
import concourse.tile as tile
from concourse import bacc
from concourse.bass import ds
from concourse.masks import make_identity

F32 = mybir.dt.float32
AF = mybir.ActivationFunctionType
ALU = mybir.AluOpType

# problem dims (per core)
B = 8          # batch per core (64 / 8 cores)
T = 512
IN = 256
H = 1024
OUT = 256
KT = H // 128   # 8 k-tiles over hidden
KI = IN // 128  # 2 k-tiles over input
NG = 4          # psum column groups
CH = H // NG    # 256 output chunk per group
SP = 3 * 32 + B  # 104 striped partitions


def _ht_slice(ht_sb, kt):
    # lhsT tile kt of a transposed-state buffer [128, 2*SP]
    # layout: block m=kt%2 at cols [m*SP, (m+1)*SP), stripe g=kt//2 at +32g
    return ht_sb[:, (kt % 2) * SP + 32 * (kt // 2):(kt % 2) * SP + 32 * (kt // 2) + B]


def build(n_steps=T, use_bias=False, unroll=8, dbg=()):
    nc = bacc.Bacc("TRN2", target_bir_lowering=False)

    X_d = nc.dram_tensor("X", [B, T, IN], F32, kind="ExternalInput")
    Wx_d = nc.dram_tensor("Wx", [H, IN], F32, kind="ExternalInput")
    Wh_d = nc.dram_tensor("Wh", [H, H], F32, kind="ExternalInput")
    Uz_d = nc.dram_tensor("Uz", [H, IN], F32, kind="ExternalInput")
    Vz_d = nc.dram_tensor("Vz", [H, H], F32, kind="ExternalInput")
    Ur_d = nc.dram_tensor("Ur", [H, IN], F32, kind="ExternalInput")
    Vr_d = nc.dram_tensor("Vr", [H, H], F32, kind="ExternalInput")
    Wo_d = nc.dram_tensor("Wo", [OUT, H], F32, kind="ExternalInput")
    if use_bias:
        bx_d = nc.dram_tensor("bx", [H], F32, kind="ExternalInput")
        bz_d = nc.dram_tensor("bz", [H], F32, kind="ExternalInput")
        br_d = nc.dram_tensor("br", [H], F32, kind="ExternalInput")
        bo_d = nc.dram_tensor("bo", [OUT], F32, kind="ExternalInput")
    Y_d = nc.dram_tensor("Y", [B, OUT], F32, kind="ExternalOutput")

    with tile.TileContext(nc) as tc:
        with tc.tile_pool(name="state", bufs=1) as st:
            # persistent SBUF tensors
            WT_h = st.tile([128, KT * H], F32, tag="WT_h")
            # r|z fused: col(kt, n) = kt*2H + (n//CH)*2CH + off + n%CH, off: r=0, z=CH
            WT_rz = st.tile([128, KT * 2 * H], F32, tag="WT_rz")
            UT_h = st.tile([128, KI * H], F32, tag="UT_h")
            UT_rz = st.tile([128, KI * 2 * H], F32, tag="UT_rz")
            WoT = st.tile([128, KT * OUT], F32, tag="WoT")
            XT = st.tile([128, T, 2 * B], F32, tag="XT")
            ident = st.tile([128, 128], F32, tag="ident")
            ones8 = st.tile([1, B], F32, tag="ones8")
            bias_sb = st.tile([1, 3 * H + OUT], F32, tag="bias_sb")
            bias_rz = st.tile([1, 2 * H], F32, tag="bias_rz")
            # striped state [SP(=104 used), 256]
            hS = st.tile([128, CH], F32, tag="hS")
            zS = st.tile([128, CH], F32, tag="zS")
            rS = st.tile([128, CH], F32, tag="rS")
            htS = st.tile([128, CH], F32, tag="htS")
            zhS = st.tile([128, CH], F32, tag="zhS")
            omzS = st.tile([128, CH], F32, tag="omzS")
            mS = st.tile([128, CH], F32, tag="mS")
            hrS = st.tile([128, CH], F32, tag="hrS")
            hT_sb = st.tile([128, 2 * SP], F32, tag="hT_sb")
            hrT_sb = st.tile([128, 2 * SP], F32, tag="hrT_sb")
            ysb = st.tile([128, OUT], F32, tag="ysb")

            make_identity(nc, ident[:])
            nc.vector.memset(ones8[:], 1.0)
            for t_ in (hS, zS, rS, htS, zhS, omzS, mS, hrS, hT_sb, hrT_sb):
                nc.vector.memset(t_[:], 0.0)
            if use_bias:
                nc.sync.dma_start(bias_sb[0, 0:H], bx_d[:])
                nc.sync.dma_start(bias_sb[0, H:2 * H], bz_d[:])
                nc.sync.dma_start(bias_sb[0, 2 * H:3 * H], br_d[:])
                nc.sync.dma_start(bias_sb[0, 3 * H:3 * H + OUT], bo_d[:])
                for g in range(NG):
                    nc.vector.tensor_copy(
                        bias_rz[0:1, g * 2 * CH:g * 2 * CH + CH],
                        bias_sb[0:1, 2 * H + g * CH:2 * H + (g + 1) * CH])
                    nc.vector.tensor_copy(
                        bias_rz[0:1, g * 2 * CH + CH:(g + 1) * 2 * CH],
                        bias_sb[0:1, H + g * CH:H + (g + 1) * CH])
            else:
                nc.vector.memset(bias_sb[:], 0.0)
                nc.vector.memset(bias_rz[:], 0.0)

            # ---------- setup: load + transpose weights ----------
            with tc.tile_pool(name="setup_sb", bufs=3) as sb_pool, \
                 tc.tile_pool(name="setup_ps", bufs=4, space="PSUM") as ps_pool:

                def transpose_into(dst, src_d, R, C, colf=None):
                    # default: dst[p, ct*R + r] = src[r, ct*128 + p]
                    if colf is None:
                        colf = lambda ct, r0: ct * R + r0
                    for rt in range(R // 128):
                        nat = sb_pool.tile([128, C], F32, tag="nat")
                        nc.sync.dma_start(nat[:, :], src_d[rt * 128:(rt + 1) * 128, :])
                        for ct in range(C // 128):
                            pt = ps_pool.tile([128, 128], F32, tag="pt")
                            nc.tensor.transpose(pt[:], nat[:, ct * 128:(ct + 1) * 128], ident[:])
                            c0 = colf(ct, rt * 128)
                            nc.vector.tensor_copy(dst[:, c0:c0 + 128], pt[:])

                def rz_col(off, gw):
                    # interleave r|z chunks of CH within each (ct, group)
                    return lambda ct, r0: ct * gw + (r0 // CH) * 2 * CH + off + (r0 % CH)

                transpose_into(WT_h, Wh_d, H, H)
                transpose_into(WT_rz, Vr_d, H, H, colf=rz_col(0, 2 * H))
                transpose_into(WT_rz, Vz_d, H, H, colf=rz_col(CH, 2 * H))
                transpose_into(UT_h, Wx_d, H, IN)
                transpose_into(UT_rz, Ur_d, H, IN, colf=rz_col(0, 2 * H))
                transpose_into(UT_rz, Uz_d, H, IN, colf=rz_col(CH, 2 * H))
                transpose_into(WoT, Wo_d, OUT, H)

                # X -> XT[p, t, ki*8+b] = X[b, t, ki*128+p]
                for b in range(B):
                    for tt in range(T // 128):
                        nat = sb_pool.tile([128, IN], F32, tag="nat")
                        nc.sync.dma_start(nat[:, :], X_d[b, tt * 128:(tt + 1) * 128, :])
                        for ki in range(KI):
                            pt = ps_pool.tile([128, 128], F32, tag="pt")
                            nc.tensor.transpose(pt[:], nat[:, ki * 128:(ki + 1) * 128], ident[:])
                            nc.vector.tensor_copy(
                                XT[:, tt * 128:(tt + 1) * 128, ki * B + b], pt[:])

            # ---------- recurrence ----------
            with tc.tile_pool(name="xp", bufs=3) as xp, \
                 tc.tile_pool(name="ps", bufs=1, space="PSUM") as ps:

                def emit_gate(pg, lhs_of_kt, WT, UT, brow, xst, cw=CH):
                    # cw: output chunk width per column-group (CH or 2*CH for fused r|z)
                    n_mm = KT + KI + (1 if use_bias else 0)
                    gw_w = gw_u = H * cw // CH  # per-k-tile rhs width
                    idx = 0
                    for kt in range(KT):
                        for g in range(NG):
                            nc.tensor.matmul(
                                pg[32 * g:32 * g + B, 0:cw],
                                lhsT=lhs_of_kt(kt),
                                rhs=WT[:, kt * gw_w + g * cw:kt * gw_w + (g + 1) * cw],
                                start=(idx == 0), stop=(idx == n_mm - 1 and g == NG - 1),
                                tile_position=(0, 32 * g))
                        idx += 1
                    for ki in range(KI):
                        for g in range(NG):
                            nc.tensor.matmul(
                                pg[32 * g:32 * g + B, 0:cw],
                                lhsT=xst[:, 0, ki * B:(ki + 1) * B],
                                rhs=UT[:, ki * gw_u + g * cw:ki * gw_u + (g + 1) * cw],
                                start=False, stop=(idx == n_mm - 1 and g == NG - 1),
                                tile_position=(0, 32 * g))
                        idx += 1
                    if use_bias:
                        for g in range(NG):
                            nc.tensor.matmul(
                                pg[32 * g:32 * g + B, 0:cw],
                                lhsT=ones8[0:1, :],
                                rhs=brow[0:1, g * cw:(g + 1) * cw],
                                start=False, stop=(g == NG - 1),
                                tile_position=(0, 32 * g))

                def transpose_pair(pg_tag, src, dst):
                    ptile = ps.tile([128, 2 * SP], F32, tag=pg_tag)
                    for m in (0, 1):
                        nc.tensor.transpose(
                            ptile[:, m * SP:(m + 1) * SP],
                            src[0:SP, m * 128:(m + 1) * 128],
                            ident[0:SP, 0:SP])
                    nc.vector.tensor_copy(dst[:], ptile[:])

                def step(t_sc):
                    xst = xp.tile([128, 1, 2 * B], F32, tag="xst")
                    nc.vector.tensor_copy(xst[:], XT[:, ds(t_sc, 1), :])
                    # off critical path: zh = z*h, omz = 1-z (previous z,h)
                    ew0 = nc.vector if "no_gpsimd" in dbg else nc.gpsimd
                    if "no_ew" not in dbg:
                        ew0.tensor_tensor(zhS[0:SP, :], zS[0:SP, :], hS[0:SP, :], ALU.mult)
                        ew0.tensor_scalar(omzS[0:SP, :], zS[0:SP, :], -1.0, 1.0, ALU.mult, ALU.add)
                    # G1 = hr @ Wh.T + x @ Wx.T (+ bx)
                    pg1 = ps.tile([128, CH], F32, tag="pg1")
                    if "no_mm" not in dbg:
                        emit_gate(pg1, lambda kt: _ht_slice(hrT_sb, kt), WT_h, UT_h,
                                  bias_sb, xst)
                    if "no_act" not in dbg:
                        nc.scalar.activation(htS[0:SP, :], pg1[0:SP, :], AF.Tanh)
                    # h' = zh + (1-z)*htilde
                    if "no_ew" not in dbg:
                        nc.vector.tensor_tensor(mS[0:SP, :], omzS[0:SP, :], htS[0:SP, :], ALU.mult)
                        nc.vector.tensor_tensor(hS[0:SP, :], zhS[0:SP, :], mS[0:SP, :], ALU.add)
                    if "no_tp" not in dbg:
                        transpose_pair("pt_h", hS, hT_sb)
                    # fused r|z gates: rhs = [Vr|Vz] interleaved per group, N=2*CH
                    pg23 = ps.tile([128, 2 * CH], F32, tag="pg23")
                    if "no_mm" not in dbg:
                        emit_gate(pg23, lambda kt: _ht_slice(hT_sb, kt), WT_rz, UT_rz,
                                  bias_rz, xst, cw=2 * CH)
                    if "no_act" not in dbg:
                        nc.scalar.activation(rS[0:SP, :], pg23[0:SP, 0:CH], AF.Sigmoid)
                        nc.scalar.activation(zS[0:SP, :], pg23[0:SP, CH:2 * CH], AF.Sigmoid)
                    # hr = h' * r ; transpose for next step
                    if "no_ew" not in dbg:
                        nc.vector.tensor_tensor(hrS[0:SP, :], hS[0:SP, :], rS[0:SP, :], ALU.mult)
                    if "no_tp" not in dbg:
                        transpose_pair("pt_hr", hrS, hrT_sb)

                if n_steps % unroll == 0 and n_steps // unroll > 1:
                    with tc.For_i(0, n_steps // unroll, 1,
                                  hint_engines=tuple(mybir.ALL_ENGINES)) as it:
                        for u in range(unroll):
                            step(it * unroll + u)
                else:
                    for t in range(n_steps):
                        step(t)

                # output: y = h @ Wo.T (+ bo)
                po = ps.tile([128, OUT], F32, tag="po")
                for kt in range(KT):
                    nc.tensor.matmul(
                        po[0:B, :], lhsT=_ht_slice(hT_sb, kt),
                        rhs=WoT[:, kt * OUT:(kt + 1) * OUT],
                        start=(kt == 0), stop=(kt == KT - 1 and not use_bias))
                if use_bias:
                    nc.tensor.matmul(
                        po[0:B, :], lhsT=ones8[0:1, :],
                        rhs=bias_sb[0:1, 3 * H:3 * H + OUT],
                        start=False, stop=True)
                nc.vector.tensor_copy(ysb[0:B, :], po[0:B, :])
                nc.sync.dma_start(Y_d[:, :], ysb[0:B, :])

    nc.compile()
    return nc


_CACHE = {}


def _get_nc(use_bias, n_steps=T, unroll=8):
    key = (use_bias, n_steps, unroll)
    if key not in _CACHE:
        _CACHE[key] = build(n_steps=n_steps, use_bias=use_bias, unroll=unroll)
    return _CACHE[key]


def kernel(**inputs):
    from concourse import bass_utils

    X = np.ascontiguousarray(inputs["X"], dtype=np.float32)
    n_cores = 8
    bt = X.shape[0] // n_cores
    use_bias = any(
        np.any(np.asarray(inputs[k]) != 0) for k in ("bx", "bz", "br", "bo") if k in inputs)
    nc = _get_nc(use_bias)

    weights = {k: np.ascontiguousarray(inputs[k], dtype=np.float32)
               for k in ("Wx", "Wh", "Uz", "Vz", "Ur", "Vr", "Wo")}
    if use_bias:
        for k in ("bx", "bz", "br", "bo"):
            weights[k] = np.ascontiguousarray(inputs[k], dtype=np.float32)

    in_maps = []
    for c in range(n_cores):
        m = dict(weights)
        m["X"] = np.ascontiguousarray(X[c * bt:(c + 1) * bt])
        in_maps.append(m)

    res = bass_utils.run_bass_kernel_spmd(nc, in_maps, core_ids=list(range(n_cores)))
    return np.concatenate([r["Y"] for r in res.results], axis=0)


if __name__ == "__main__":
    nc = build(n_steps=int(os.environ.get("STEPS", "16")), unroll=8)
    print("build OK")



# revision 23
# speedup vs baseline: 21.9981x; 21.9981x over previous
"""Trainium2 Bass kernel for a nonstandard GRU (gates computed after state update).

Strategy: data-parallel over batch (64 samples -> 8 cores x 8 samples).
Per core, the T=512 sequential recurrence runs entirely from SBUF with the
matmuls in weights-stationary orientation:
  - each gate matmul is out[128-chunk of H, B=8] = W_chunk^T.T @ h_chunk,
    i.e. lhsT = weight tile [K=128, M=128] (stationary), rhs = state
    [K=128, N=8] (moving) -> only 8 PE rows per matmul instruction.
  - gate outputs land in PSUM as [128, 8] tiles laid out side by side
    ([128, 64] per gate), which IS the transposed state layout the next
    matmul needs as rhs -> no PE transposes anywhere in the loop.
  - input projections (X @ Wx^T etc.) are folded in as 2 extra K-tiles
    from a pre-transposed XT; they are issued first so they fill PE gaps
    while the tanh/sigmoid/elementwise chain of the previous phase runs.
  - elementwise/activation ops see [128 partitions, 64 free] tensors.
"""

import os
import sys

sys.path.insert(0, "/opt/trn_rl_repo")

import numpy as np

import concourse.bass as bass
import concourse.mybir as mybir
import concourse.tile as tile
from concourse import bacc
from concourse.bass import ds

F32 = mybir.dt.float32
F16 = mybir.dt.float16  # matmul operands: 1 cycle/row (vs 4 for fp32), fp32 PSUM accum
AF = mybir.ActivationFunctionType
ALU = mybir.AluOpType

# problem dims (per core)
B = 8          # batch per core (64 / 8 cores)
T = 512
IN = 256
H = 1024
OUT = 256
KT = H // 128   # 8 k-tiles / out-tiles over hidden
KI = IN // 128  # 2 k-tiles over input
SW = KT * B     # 64: state width in transposed layout [128, SW]


def build(n_steps=T, use_bias=False, unroll=8, dbg=()):
    nc = bacc.Bacc("TRN2", target_bir_lowering=False)

    # All inputs are pre-transposed + fp16-converted on the HOST (see
    # _prep_weights/_prep_x below); device setup is then just straight DMAs.
    XT_d = nc.dram_tensor("XT", [128, T, KI * B], F16, kind="ExternalInput")
    WhT_d = nc.dram_tensor("WhT", [128, KT * H], F16, kind="ExternalInput")
    VzT_d = nc.dram_tensor("VzT", [128, KT * H], F16, kind="ExternalInput")
    VrT_d = nc.dram_tensor("VrT", [128, KT * H], F16, kind="ExternalInput")
    WxT_d = nc.dram_tensor("WxT", [128, KI * H], F16, kind="ExternalInput")
    UzT_d = nc.dram_tensor("UzT", [128, KI * H], F16, kind="ExternalInput")
    UrT_d = nc.dram_tensor("UrT", [128, KI * H], F16, kind="ExternalInput")
    WoT_d = nc.dram_tensor("WoT", [128, KT * OUT], F16, kind="ExternalInput")
    if use_bias:
        bias_d = nc.dram_tensor("biases", [1, 3 * H + OUT], F16, kind="ExternalInput")
    Y_d = nc.dram_tensor("Y", [B, OUT], F32, kind="ExternalOutput")

    with tile.TileContext(nc) as tc:
        with tc.tile_pool(name="state", bufs=1) as st:
            # persistent SBUF tensors
            # weight layouts: WT[p, kt*H + c] = W[c, kt*128 + p]
            #   -> lhsT(kt, mt) = WT[:, kt*H + mt*128 :][:128] is a [K=128, M=128]
            #      stationary tile of W^T
            WT_h = st.tile([128, KT * H], F16, tag="WT_h")
            VzT = st.tile([128, KT * H], F16, tag="VzT")
            VrT = st.tile([128, KT * H], F16, tag="VrT")
            UT_h = st.tile([128, KI * H], F16, tag="UT_h")
            UzT = st.tile([128, KI * H], F16, tag="UzT")
            UrT = st.tile([128, KI * H], F16, tag="UrT")
            WoT = st.tile([128, KT * OUT], F16, tag="WoT")
            XT = st.tile([128, T, KI * B], F16, tag="XT")
            ones8 = st.tile([1, B], F16, tag="ones8")
            bias_sb = st.tile([1, 3 * H + OUT], F16, tag="bias_sb")
            # transposed state [128, SW]: col ct*B + b <-> state[b, ct*128 + p]
            hT = st.tile([128, SW], F16, tag="hT")
            zT = st.tile([128, SW], F16, tag="zT")
            rT = st.tile([128, SW], F16, tag="rT")
            htT = st.tile([128, SW], F16, tag="htT")
            zhT = st.tile([128, SW], F16, tag="zhT")
            omzT = st.tile([128, SW], F16, tag="omzT")
            mT = st.tile([128, SW], F16, tag="mT")
            hrT = st.tile([128, SW], F16, tag="hrT")
            ysb = st.tile([128, OUT], F32, tag="ysb")

            nc.vector.memset(ones8[:], 1.0)
            for t_ in (hT, zT, rT, htT, zhT, omzT, mT, hrT):
                nc.vector.memset(t_[:], 0.0)
            if use_bias:
                nc.sync.dma_start(bias_sb[:, :], bias_d[:, :])
            else:
                nc.vector.memset(bias_sb[:], 0.0)

            # ---------- setup: straight DMAs of host-pre-transposed data ----
            # ordered by first use in the recurrence (WoT only needed at the
            # very end) so step 0 can start before the tail DMAs land
            nc.sync.dma_start(UT_h[:, :], WxT_d[:, :])
            nc.sync.dma_start(UzT[:, :], UzT_d[:, :])
            nc.sync.dma_start(UrT[:, :], UrT_d[:, :])
            nc.sync.dma_start(XT[:, 0:T // 4, :], XT_d[:, 0:T // 4, :])
            nc.sync.dma_start(WT_h[:, :], WhT_d[:, :])
            nc.sync.dma_start(VrT[:, :], VrT_d[:, :])
            nc.sync.dma_start(VzT[:, :], VzT_d[:, :])
            nc.sync.dma_start(XT[:, T // 4:T, :], XT_d[:, T // 4:T, :])
            nc.sync.dma_start(WoT[:, :], WoT_d[:, :])

            # ---------- recurrence ----------
            with tc.tile_pool(name="xp", bufs=3) as xp, \
                 tc.tile_pool(name="ps", bufs=1, space="PSUM") as ps:

                # PSUM start/stop semantics: start=True on the FIRST matmul
                # marks the whole 2KB zero region pending-zero; every later
                # matmul (start=False) zero-initializes the bytes it is
                # first to touch and accumulates thereafter. One group per
                # gate per bank-aligned psum tile. x-projection k-tiles are
                # issued first (they depend only on xst) so they fill PE gaps
                # while the previous phase's act/elementwise chain runs.
                def emit_xproj(pg, UT, boff, xs):
                    for mt in range(KT):
                        o = mt * B
                        for ki in range(KI):
                            nc.tensor.matmul(
                                pg[:, o:o + B],
                                lhsT=UT[:, ki * H + mt * 128:ki * H + mt * 128 + 128],
                                rhs=xs[ki],
                                start=(mt == 0 and ki == 0), stop=False)
                        if use_bias:
                            nc.tensor.matmul(
                                pg[:, o:o + B],
                                lhsT=bias_sb[0:1, boff + mt * 128:boff + (mt + 1) * 128],
                                rhs=ones8[0:1, :],
                                start=False, stop=False)

                def emit_rec(pg, WT, hsrc, last=True):
                    for kt in range(KT):
                        for mt in range(KT):
                            o = mt * B
                            nc.tensor.matmul(
                                pg[:, o:o + B],
                                lhsT=WT[:, kt * H + mt * 128:kt * H + mt * 128 + 128],
                                rhs=hsrc[:, kt * B:(kt + 1) * B],
                                start=False,
                                stop=(last and kt == KT - 1 and mt == KT - 1))

                def step(t_sc):
                    xst = xp.tile([128, 1, KI * B], F16, tag="xst")
                    # DVE beats gpsimd here: no Q7 launch cost, and the chain
                    # ops that consume zh/omz are on DVE anyway (no sem hop)
                    ew0 = nc.gpsimd if "use_gpsimd" in dbg else nc.vector
                    ew0.tensor_copy(xst[:], XT[:, ds(t_sc, 1), :])
                    xs = [xst[:, 0, ki * B:(ki + 1) * B] for ki in range(KI)]
                    # off critical path: zh = z*h, omz = 1-z (previous z,h)
                    if "no_ew" not in dbg:
                        ew0.tensor_tensor(zhT[:, :], zT[:, :], hT[:, :], ALU.mult)
                        ew0.tensor_scalar(omzT[:, :], zT[:, :], -1.0, 1.0, ALU.mult, ALU.add)
                    # V.h' is split: V.zh streams early (zh is ready at step
                    # start), only q = (1-z)*tanh(G1) stays on the chain, and
                    # h' = zh + q forms off-cycle (needed for hr + next zh).
                    pg1 = ps.tile([128, 512], F32, tag="pg1")
                    pgr = ps.tile([128, 512], F32, tag="pgr")
                    pgz = ps.tile([128, 512], F32, tag="pgz")
                    if "no_mm" not in dbg:
                        emit_xproj(pg1, UT_h, 0, xs)
                        emit_xproj(pgr, UrT, 2 * H, xs)
                        emit_xproj(pgz, UzT, H, xs)
                        emit_rec(pg1, WT_h, hrT)          # on-cycle (hr_{t-1})
                        emit_rec(pgr, VrT, zhT, last=False)  # fill: tanh window
                        emit_rec(pgz, VzT, zhT, last=False)
                    if "no_act" not in dbg:
                        nc.scalar.activation(htT[:, :], pg1[:, 0:SW], AF.Tanh)
                    if "no_ew" not in dbg:
                        nc.vector.tensor_tensor(mT[:, :], omzT[:, :], htT[:, :], ALU.mult)
                        nc.vector.tensor_tensor(hT[:, :], zhT[:, :], mT[:, :], ALU.add)
                    if "no_mm" not in dbg:
                        emit_rec(pgr, VrT, mT)            # on-cycle (q)
                        emit_rec(pgz, VzT, mT)            # fills sigmoid window
                    if "no_act" not in dbg:
                        nc.scalar.activation(rT[:, :], pgr[:, 0:SW], AF.Sigmoid)
                    if "no_ew" not in dbg:
                        nc.vector.tensor_tensor(hrT[:, :], hT[:, :], rT[:, :], ALU.mult)
                    if "no_act" not in dbg:
                        nc.scalar.activation(zT[:, :], pgz[:, 0:SW], AF.Sigmoid)

                if n_steps % unroll == 0 and n_steps // unroll > 1:
                    with tc.For_i(0, n_steps // unroll, 1,
                                  hint_engines=tuple(mybir.ALL_ENGINES)) as it:
                        for u in range(unroll):
                            step(it * unroll + u)
                else:
                    for t in range(n_steps):
                        step(t)

                # output: y = h @ Wo.T (+ bo); hT is already in lhsT layout
                po = ps.tile([128, 512], F32, tag="po")
                for kt in range(KT):
                    nc.tensor.matmul(
                        po[0:B, 0:OUT], lhsT=hT[:, kt * B:(kt + 1) * B],
                        rhs=WoT[:, kt * OUT:(kt + 1) * OUT],
                        start=(kt == 0), stop=(kt == KT - 1 and not use_bias))
                if use_bias:
                    nc.tensor.matmul(
                        po[0:B, 0:OUT], lhsT=ones8[0:1, :],
                        rhs=bias_sb[0:1, 3 * H:3 * H + OUT],
                        start=False, stop=True)
                nc.vector.tensor_copy(ysb[0:B, :], po[0:B, 0:OUT])
                nc.sync.dma_start(Y_d[:, :], ysb[0:B, :])

    nc.compile()
    return nc


_CACHE = {}


def _get_nc(use_bias, n_steps=T, unroll=8):
    key = (use_bias, n_steps, unroll)
    if key not in _CACHE:
        _CACHE[key] = build(n_steps=n_steps, use_bias=use_bias, unroll=unroll)
    return _CACHE[key]


def _wt(W):
    # W [R, C] -> WT [128, (C//128) * R] fp16 with WT[p, kt*R + r] = W[r, kt*128 + p]
    R, C = W.shape
    return np.ascontiguousarray(
        W.T.reshape(C // 128, 128, R).transpose(1, 0, 2).reshape(128, -1),
        dtype=np.float16)


def prep_in_maps(inputs, n_cores=8):
    X = np.asarray(inputs["X"], dtype=np.float32)
    bt = X.shape[0] // n_cores
    use_bias = any(
        np.any(np.asarray(inputs[k]) != 0) for k in ("bx", "bz", "br", "bo") if k in inputs)

    weights = {
        "WhT": _wt(np.asarray(inputs["Wh"], np.float32)),
        "VzT": _wt(np.asarray(inputs["Vz"], np.float32)),
        "VrT": _wt(np.asarray(inputs["Vr"], np.float32)),
        "WxT": _wt(np.asarray(inputs["Wx"], np.float32)),
        "UzT": _wt(np.asarray(inputs["Uz"], np.float32)),
        "UrT": _wt(np.asarray(inputs["Ur"], np.float32)),
        "WoT": _wt(np.asarray(inputs["Wo"], np.float32)),
    }
    if use_bias:
        weights["biases"] = np.concatenate(
            [np.asarray(inputs[k], np.float32) for k in ("bx", "bz", "br", "bo")]
        ).reshape(1, -1).astype(np.float16)

    in_maps = []
    for c in range(n_cores):
        m = dict(weights)
        Xc = X[c * bt:(c + 1) * bt]  # [B, T, IN]
        # XT[p, t, ki*B + b] = X[b, t, ki*128 + p]
        m["XT"] = np.ascontiguousarray(
            Xc.reshape(bt, T, KI, 128).transpose(3, 1, 2, 0).reshape(128, T, KI * bt),
            dtype=np.float16)
        in_maps.append(m)
    return in_maps, use_bias


def kernel(**inputs):
    from concourse import bass_utils

    n_cores = 8
    in_maps, use_bias = prep_in_maps(inputs, n_cores)
    nc = _get_nc(use_bias)
    res = bass_utils.run_bass_kernel_spmd(nc, in_maps, core_ids=list(range(n_cores)))
    return np.concatenate([r["Y"] for r in res.results], axis=0)


if __name__ == "__main__":
    nc = build(n_steps=int(os.environ.get("STEPS", "16")), unroll=8)
    print("build OK")


# revision 24
# speedup vs baseline: 22.0105x; 1.0006x over previous
"""Trainium2 Bass kernel for a nonstandard GRU (gates computed after state update).

Strategy: data-parallel over batch (64 samples -> 8 cores x 8 samples).
Per core, the T=512 sequential recurrence runs entirely from SBUF with the
matmuls in weights-stationary orientation:
  - each gate matmul is out[128-chunk of H, B=8] = W_chunk^T.T @ h_chunk,
    i.e. lhsT = weight tile [K=128, M=128] (stationary), rhs = state
    [K=128, N=8] (moving) -> only 8 PE rows per matmul instruction.
  - gate outputs land in PSUM as [128, 8] tiles laid out side by side
    ([128, 64] per gate), which IS the transposed state layout the next
    matmul needs as rhs -> no PE transposes anywhere in the loop.
  - input projections (X @ Wx^T etc.) are folded in as 2 extra K-tiles
    from a pre-transposed XT; they are issued first so they fill PE gaps
    while the tanh/sigmoid/elementwise chain of the previous phase runs.
  - elementwise/activation ops see [128 partitions, 64 free] tensors.
"""

import os
import sys

sys.path.insert(0, "/opt/trn_rl_repo")

import numpy as np

import concourse.bass as bass
import concourse.mybir as mybir
import concourse.tile as tile
from concourse import bacc
from concourse.bass import ds

F32 = mybir.dt.float32
F16 = mybir.dt.float16  # matmul operands: 1 cycle/row (vs 4 for fp32), fp32 PSUM accum
AF = mybir.ActivationFunctionType
ALU = mybir.AluOpType

# problem dims (per core)
B = 8          # batch per core (64 / 8 cores)
T = 512
IN = 256
H = 1024
OUT = 256
KT = H // 128   # 8 k-tiles / out-tiles over hidden
KI = IN // 128  # 2 k-tiles over input
SW = KT * B     # 64: state width in transposed layout [128, SW]


def build(n_steps=T, use_bias=False, unroll=8, dbg=()):
    nc = bacc.Bacc("TRN2", target_bir_lowering=False)

    # All inputs are pre-transposed + fp16-converted on the HOST (see
    # _prep_weights/_prep_x below); device setup is then just straight DMAs.
    XT_d = nc.dram_tensor("XT", [128, T, KI * B], F16, kind="ExternalInput")
    WhT_d = nc.dram_tensor("WhT", [128, KT * H], F16, kind="ExternalInput")
    VzT_d = nc.dram_tensor("VzT", [128, KT * H], F16, kind="ExternalInput")
    VrT_d = nc.dram_tensor("VrT", [128, KT * H], F16, kind="ExternalInput")
    WxT_d = nc.dram_tensor("WxT", [128, KI * H], F16, kind="ExternalInput")
    UzT_d = nc.dram_tensor("UzT", [128, KI * H], F16, kind="ExternalInput")
    UrT_d = nc.dram_tensor("UrT", [128, KI * H], F16, kind="ExternalInput")
    WoT_d = nc.dram_tensor("WoT", [128, KT * OUT], F16, kind="ExternalInput")
    if use_bias:
        bias_d = nc.dram_tensor("biases", [1, 3 * H + OUT], F16, kind="ExternalInput")
    Y_d = nc.dram_tensor("Y", [B, OUT], F32, kind="ExternalOutput")

    with tile.TileContext(nc) as tc:
        with tc.tile_pool(name="state", bufs=1) as st:
            # persistent SBUF tensors
            # weight layouts: WT[p, kt*H + c] = W[c, kt*128 + p]
            #   -> lhsT(kt, mt) = WT[:, kt*H + mt*128 :][:128] is a [K=128, M=128]
            #      stationary tile of W^T
            WT_h = st.tile([128, KT * H], F16, tag="WT_h")
            VzT = st.tile([128, KT * H], F16, tag="VzT")
            VrT = st.tile([128, KT * H], F16, tag="VrT")
            UT_h = st.tile([128, KI * H], F16, tag="UT_h")
            UzT = st.tile([128, KI * H], F16, tag="UzT")
            UrT = st.tile([128, KI * H], F16, tag="UrT")
            WoT = st.tile([128, KT * OUT], F16, tag="WoT")
            XT = st.tile([128, T, KI * B], F16, tag="XT")
            ones8 = st.tile([1, B], F16, tag="ones8")
            bias_sb = st.tile([1, 3 * H + OUT], F16, tag="bias_sb")
            # transposed state [128, SW]: col ct*B + b <-> state[b, ct*128 + p]
            hT = st.tile([128, SW], F16, tag="hT")
            zT = st.tile([128, SW], F16, tag="zT")
            rT = st.tile([128, SW], F16, tag="rT")
            htT = st.tile([128, SW], F16, tag="htT")
            zhT = st.tile([128, SW], F16, tag="zhT")
            omzT = st.tile([128, SW], F16, tag="omzT")
            mT = st.tile([128, SW], F16, tag="mT")
            hrT = st.tile([128, SW], F16, tag="hrT")
            ysb = st.tile([128, OUT], F32, tag="ysb")

            nc.vector.memset(ones8[:], 1.0)
            for t_ in (hT, zT, rT, htT, zhT, omzT, mT, hrT):
                nc.vector.memset(t_[:], 0.0)
            if use_bias:
                nc.sync.dma_start(bias_sb[:, :], bias_d[:, :])
            else:
                nc.vector.memset(bias_sb[:], 0.0)

            # ---------- setup: straight DMAs of host-pre-transposed data ----
            # ordered by first use in the recurrence (WoT only needed at the
            # very end) so step 0 can start before the tail DMAs land
            nc.sync.dma_start(XT[:, 0:T // 8, :], XT_d[:, 0:T // 8, :])
            nc.sync.dma_start(UT_h[:, :], WxT_d[:, :])
            nc.sync.dma_start(UzT[:, :], UzT_d[:, :])
            nc.sync.dma_start(UrT[:, :], UrT_d[:, :])
            nc.sync.dma_start(WT_h[:, :], WhT_d[:, :])
            nc.sync.dma_start(VrT[:, :], VrT_d[:, :])
            nc.sync.dma_start(VzT[:, :], VzT_d[:, :])
            nc.sync.dma_start(XT[:, T // 8:T, :], XT_d[:, T // 8:T, :])
            nc.sync.dma_start(WoT[:, :], WoT_d[:, :])

            # ---------- recurrence ----------
            with tc.tile_pool(name="xp", bufs=3) as xp, \
                 tc.tile_pool(name="ps", bufs=1, space="PSUM") as ps:

                # PSUM start/stop semantics: start=True on the FIRST matmul
                # marks the whole 2KB zero region pending-zero; every later
                # matmul (start=False) zero-initializes the bytes it is
                # first to touch and accumulates thereafter. One group per
                # gate per bank-aligned psum tile. x-projection k-tiles are
                # issued first (they depend only on xst) so they fill PE gaps
                # while the previous phase's act/elementwise chain runs.
                def emit_xproj(pg, UT, boff, xs):
                    for mt in range(KT):
                        o = mt * B
                        for ki in range(KI):
                            nc.tensor.matmul(
                                pg[:, o:o + B],
                                lhsT=UT[:, ki * H + mt * 128:ki * H + mt * 128 + 128],
                                rhs=xs[ki],
                                start=(mt == 0 and ki == 0), stop=False)
                        if use_bias:
                            nc.tensor.matmul(
                                pg[:, o:o + B],
                                lhsT=bias_sb[0:1, boff + mt * 128:boff + (mt + 1) * 128],
                                rhs=ones8[0:1, :],
                                start=False, stop=False)

                def emit_rec(pg, WT, hsrc, last=True):
                    for kt in range(KT):
                        for mt in range(KT):
                            o = mt * B
                            nc.tensor.matmul(
                                pg[:, o:o + B],
                                lhsT=WT[:, kt * H + mt * 128:kt * H + mt * 128 + 128],
                                rhs=hsrc[:, kt * B:(kt + 1) * B],
                                start=False,
                                stop=(last and kt == KT - 1 and mt == KT - 1))

                def step(t_sc):
                    xst = xp.tile([128, 1, KI * B], F16, tag="xst")
                    # DVE beats gpsimd here: no Q7 launch cost, and the chain
                    # ops that consume zh/omz are on DVE anyway (no sem hop)
                    ew0 = nc.gpsimd if "use_gpsimd" in dbg else nc.vector
                    ew0.tensor_copy(xst[:], XT[:, ds(t_sc, 1), :])
                    xs = [xst[:, 0, ki * B:(ki + 1) * B] for ki in range(KI)]
                    # off critical path: zh = z*h, omz = 1-z (previous z,h)
                    if "no_ew" not in dbg:
                        ew0.tensor_tensor(zhT[:, :], zT[:, :], hT[:, :], ALU.mult)
                        ew0.tensor_scalar(omzT[:, :], zT[:, :], -1.0, 1.0, ALU.mult, ALU.add)
                    # V.h' is split: V.zh streams early (zh is ready at step
                    # start), only q = (1-z)*tanh(G1) stays on the chain, and
                    # h' = zh + q forms off-cycle (needed for hr + next zh).
                    pg1 = ps.tile([128, 512], F32, tag="pg1")
                    pgr = ps.tile([128, 512], F32, tag="pgr")
                    pgz = ps.tile([128, 512], F32, tag="pgz")
                    if "no_mm" not in dbg:
                        emit_xproj(pg1, UT_h, 0, xs)
                        emit_xproj(pgr, UrT, 2 * H, xs)
                        emit_xproj(pgz, UzT, H, xs)
                        emit_rec(pg1, WT_h, hrT)          # on-cycle (hr_{t-1})
                        emit_rec(pgr, VrT, zhT, last=False)  # fill: tanh window
                        emit_rec(pgz, VzT, zhT, last=False)
                    if "no_act" not in dbg:
                        nc.scalar.activation(htT[:, :], pg1[:, 0:SW], AF.Tanh)
                    if "no_ew" not in dbg:
                        nc.vector.tensor_tensor(mT[:, :], omzT[:, :], htT[:, :], ALU.mult)
                        nc.vector.tensor_tensor(hT[:, :], zhT[:, :], mT[:, :], ALU.add)
                    if "no_mm" not in dbg:
                        emit_rec(pgr, VrT, mT)            # on-cycle (q)
                        emit_rec(pgz, VzT, mT)            # fills sigmoid window
                    if "no_act" not in dbg:
                        nc.scalar.activation(rT[:, :], pgr[:, 0:SW], AF.Sigmoid)
                    if "no_ew" not in dbg:
                        nc.vector.tensor_tensor(hrT[:, :], hT[:, :], rT[:, :], ALU.mult)
                    if "no_act" not in dbg:
                        nc.scalar.activation(zT[:, :], pgz[:, 0:SW], AF.Sigmoid)

                if n_steps % unroll == 0 and n_steps // unroll > 1:
                    with tc.For_i(0, n_steps // unroll, 1,
                                  hint_engines=tuple(mybir.ALL_ENGINES)) as it:
                        for u in range(unroll):
                            step(it * unroll + u)
                else:
                    for t in range(n_steps):
                        step(t)

                # output: y = h @ Wo.T (+ bo); hT is already in lhsT layout
                po = ps.tile([128, 512], F32, tag="po")
                for kt in range(KT):
                    nc.tensor.matmul(
                        po[0:B, 0:OUT], lhsT=hT[:, kt * B:(kt + 1) * B],
                        rhs=WoT[:, kt * OUT:(kt + 1) * OUT],
                        start=(kt == 0), stop=(kt == KT - 1 and not use_bias))
                if use_bias:
                    nc.tensor.matmul(
                        po[0:B, 0:OUT], lhsT=ones8[0:1, :],
                        rhs=bias_sb[0:1, 3 * H:3 * H + OUT],
                        start=False, stop=True)
                nc.vector.tensor_copy(ysb[0:B, :], po[0:B, 0:OUT])
                nc.sync.dma_start(Y_d[:, :], ysb[0:B, :])

    nc.compile()
    return nc


_CACHE = {}


def _get_nc(use_bias, n_steps=T, unroll=8):
    key = (use_bias, n_steps, unroll)
    if key not in _CACHE:
        _CACHE[key] = build(n_steps=n_steps, use_bias=use_bias, unroll=unroll)
    return _CACHE[key]


def _wt(W):
    # W [R, C] -> WT [128, (C//128) * R] fp16 with WT[p, kt*R + r] = W[r, kt*128 + p]
    R, C = W.shape
    return np.ascontiguousarray(
        W.T.reshape(C // 128, 128, R).transpose(1, 0, 2).reshape(128, -1),
        dtype=np.float16)


def prep_in_maps(inputs, n_cores=8):
    X = np.asarray(inputs["X"], dtype=np.float32)
    bt = X.shape[0] // n_cores
    use_bias = any(
        np.any(np.asarray(inputs[k]) != 0) for k in ("bx", "bz", "br", "bo") if k in inputs)

    weights = {
        "WhT": _wt(np.asarray(inputs["Wh"], np.float32)),
        "VzT": _wt(np.asarray(inputs["Vz"], np.float32)),
        "VrT": _wt(np.asarray(inputs["Vr"], np.float32)),
        "WxT": _wt(np.asarray(inputs["Wx"], np.float32)),
        "UzT": _wt(np.asarray(inputs["Uz"], np.float32)),
        "UrT": _wt(np.asarray(inputs["Ur"], np.float32)),
        "WoT": _wt(np.asarray(inputs["Wo"], np.float32)),
    }
    if use_bias:
        weights["biases"] = np.concatenate(
            [np.asarray(inputs[k], np.float32) for k in ("bx", "bz", "br", "bo")]
        ).reshape(1, -1).astype(np.float16)

    in_maps = []
    for c in range(n_cores):
        m = dict(weights)
        Xc = X[c * bt:(c + 1) * bt]  # [B, T, IN]
        # XT[p, t, ki*B + b] = X[b, t, ki*128 + p]
        m["XT"] = np.ascontiguousarray(
            Xc.reshape(bt, T, KI, 128).transpose(3, 1, 2, 0).reshape(128, T, KI * bt),
            dtype=np.float16)
        in_maps.append(m)
    return in_maps, use_bias


def kernel(**inputs):
    from concourse import bass_utils

    n_cores = 8
    in_maps, use_bias = prep_in_maps(inputs, n_cores)
    nc = _get_nc(use_bias)
    res = bass_utils.run_bass_kernel_spmd(nc, in_maps, core_ids=list(range(n_cores)))
    return np.concatenate([r["Y"] for r in res.results], axis=0)


if __name__ == "__main__":
    nc = build(n_steps=int(os.environ.get("STEPS", "16")), unroll=8)
    print("build OK")


# revision 26
# speedup vs baseline: 22.0172x; 1.0003x over previous
"""Trainium2 Bass kernel for a nonstandard GRU (gates computed after state update).

Strategy: data-parallel over batch (64 samples -> 8 cores x 8 samples).
Per core, the T=512 sequential recurrence runs entirely from SBUF with the
matmuls in weights-stationary orientation:
  - each gate matmul is out[128-chunk of H, B=8] = W_chunk^T.T @ h_chunk,
    i.e. lhsT = weight tile [K=128, M=128] (stationary), rhs = state
    [K=128, N=8] (moving) -> only 8 PE rows per matmul instruction.
  - gate outputs land in PSUM as [128, 8] tiles laid out side by side
    ([128, 64] per gate), which IS the transposed state layout the next
    matmul needs as rhs -> no PE transposes anywhere in the loop.
  - input projections (X @ Wx^T etc.) are folded in as 2 extra K-tiles
    from a pre-transposed XT; they are issued first so they fill PE gaps
    while the tanh/sigmoid/elementwise chain of the previous phase runs.
  - elementwise/activation ops see [128 partitions, 64 free] tensors.
"""

import os
import sys

sys.path.insert(0, "/opt/trn_rl_repo")

import numpy as np

import concourse.bass as bass
import concourse.mybir as mybir
import concourse.tile as tile
from concourse import bacc
from concourse.bass import ds

F32 = mybir.dt.float32
F16 = mybir.dt.float16  # matmul operands: 1 cycle/row (vs 4 for fp32), fp32 PSUM accum
AF = mybir.ActivationFunctionType
ALU = mybir.AluOpType

# problem dims (per core)
B = 8          # batch per core (64 / 8 cores)
T = 512
IN = 256
H = 1024
OUT = 256
KT = H // 128   # 8 k-tiles / out-tiles over hidden
KI = IN // 128  # 2 k-tiles over input
SW = KT * B     # 64: state width in transposed layout [128, SW]


def build(n_steps=T, use_bias=False, unroll=8, dbg=()):
    nc = bacc.Bacc("TRN2", target_bir_lowering=False)

    # All inputs are pre-transposed + fp16-converted on the HOST (see
    # _prep_weights/_prep_x below); device setup is then just straight DMAs.
    XT_d = nc.dram_tensor("XT", [128, T, KI * B], F16, kind="ExternalInput")
    WhT_d = nc.dram_tensor("WhT", [128, KT * H], F16, kind="ExternalInput")
    VzT_d = nc.dram_tensor("VzT", [128, KT * H], F16, kind="ExternalInput")
    VrT_d = nc.dram_tensor("VrT", [128, KT * H], F16, kind="ExternalInput")
    WxT_d = nc.dram_tensor("WxT", [128, KI * H], F16, kind="ExternalInput")
    UzT_d = nc.dram_tensor("UzT", [128, KI * H], F16, kind="ExternalInput")
    UrT_d = nc.dram_tensor("UrT", [128, KI * H], F16, kind="ExternalInput")
    WoT_d = nc.dram_tensor("WoT", [128, KT * OUT], F16, kind="ExternalInput")
    if use_bias:
        bias_d = nc.dram_tensor("biases", [1, 3 * H + OUT], F16, kind="ExternalInput")
    Y_d = nc.dram_tensor("Y", [B, OUT], F32, kind="ExternalOutput")

    with tile.TileContext(nc) as tc:
        with tc.tile_pool(name="state", bufs=1) as st:
            # persistent SBUF tensors
            # weight layouts: WT[p, kt*H + c] = W[c, kt*128 + p]
            #   -> lhsT(kt, mt) = WT[:, kt*H + mt*128 :][:128] is a [K=128, M=128]
            #      stationary tile of W^T
            WT_h = st.tile([128, KT * H], F16, tag="WT_h")
            VzT = st.tile([128, KT * H], F16, tag="VzT")
            VrT = st.tile([128, KT * H], F16, tag="VrT")
            UT_h = st.tile([128, KI * H], F16, tag="UT_h")
            UzT = st.tile([128, KI * H], F16, tag="UzT")
            UrT = st.tile([128, KI * H], F16, tag="UrT")
            WoT = st.tile([128, KT * OUT], F16, tag="WoT")
            XT = st.tile([128, T, KI * B], F16, tag="XT")
            ones8 = st.tile([1, B], F16, tag="ones8")
            bias_sb = st.tile([1, 3 * H + OUT], F16, tag="bias_sb")
            # transposed state [128, SW]: col ct*B + b <-> state[b, ct*128 + p]
            hT = st.tile([128, SW], F16, tag="hT")
            zT = st.tile([128, SW], F16, tag="zT")
            rT = st.tile([128, SW], F16, tag="rT")
            htT = st.tile([128, SW], F16, tag="htT")
            zhT = st.tile([128, SW], F16, tag="zhT")
            omzT = st.tile([128, SW], F16, tag="omzT")
            mT = st.tile([128, SW], F16, tag="mT")
            hrT = st.tile([128, SW], F16, tag="hrT")
            ysb = st.tile([128, OUT], F32, tag="ysb")

            nc.vector.memset(ones8[:], 1.0)
            for t_ in (hT, zT, rT, htT, zhT, omzT, mT, hrT):
                nc.vector.memset(t_[:], 0.0)
            if use_bias:
                nc.sync.dma_start(bias_sb[:, :], bias_d[:, :])
            else:
                nc.vector.memset(bias_sb[:], 0.0)

            # ---------- setup: straight DMAs of host-pre-transposed data ----
            # ordered by first use in the recurrence (WoT only needed at the
            # very end) so step 0 can start before the tail DMAs land
            nc.sync.dma_start(XT[:, 0:T // 8, :], XT_d[:, 0:T // 8, :])
            nc.sync.dma_start(UT_h[:, :], WxT_d[:, :])
            nc.sync.dma_start(UzT[:, :], UzT_d[:, :])
            nc.sync.dma_start(UrT[:, :], UrT_d[:, :])
            nc.sync.dma_start(WT_h[:, :], WhT_d[:, :])
            nc.sync.dma_start(VrT[:, :], VrT_d[:, :])
            nc.sync.dma_start(VzT[:, :], VzT_d[:, :])
            nc.sync.dma_start(XT[:, T // 8:T, :], XT_d[:, T // 8:T, :])
            nc.sync.dma_start(WoT[:, :], WoT_d[:, :])

            # ---------- recurrence ----------
            with tc.tile_pool(name="xp", bufs=3) as xp, \
                 tc.tile_pool(name="ps", bufs=1, space="PSUM") as ps:

                # PSUM start/stop semantics: start=True on the FIRST matmul
                # marks the whole 2KB zero region pending-zero; every later
                # matmul (start=False) zero-initializes the bytes it is
                # first to touch and accumulates thereafter. One group per
                # gate per bank-aligned psum tile. x-projection k-tiles are
                # issued first (they depend only on xst) so they fill PE gaps
                # while the previous phase's act/elementwise chain runs.
                def emit_xproj(pg, UT, boff, xs):
                    for mt in range(KT):
                        o = mt * B
                        for ki in range(KI):
                            nc.tensor.matmul(
                                pg[:, o:o + B],
                                lhsT=UT[:, ki * H + mt * 128:ki * H + mt * 128 + 128],
                                rhs=xs[ki],
                                start=(mt == 0 and ki == 0), stop=False)
                        if use_bias:
                            nc.tensor.matmul(
                                pg[:, o:o + B],
                                lhsT=bias_sb[0:1, boff + mt * 128:boff + (mt + 1) * 128],
                                rhs=ones8[0:1, :],
                                start=False, stop=False)

                def emit_rec(pg, WT, hsrc, last=True):
                    for kt in range(KT):
                        for mt in range(KT):
                            o = mt * B
                            nc.tensor.matmul(
                                pg[:, o:o + B],
                                lhsT=WT[:, kt * H + mt * 128:kt * H + mt * 128 + 128],
                                rhs=hsrc[:, kt * B:(kt + 1) * B],
                                start=False,
                                stop=(last and kt == KT - 1 and mt == KT - 1))

                def step(t_sc, last=False):
                    # last step: the r/z gates are dead (output needs only h),
                    # so skip their matmul streams, sigmoids, and hr
                    xst = xp.tile([128, 1, KI * B], F16, tag="xst")
                    # DVE beats gpsimd here: no Q7 launch cost, and the chain
                    # ops that consume zh/omz are on DVE anyway (no sem hop)
                    ew0 = nc.gpsimd if "use_gpsimd" in dbg else nc.vector
                    ew0.tensor_copy(xst[:], XT[:, ds(t_sc, 1), :])
                    xs = [xst[:, 0, ki * B:(ki + 1) * B] for ki in range(KI)]
                    # off critical path: zh = z*h, omz = 1-z (previous z,h)
                    if "no_ew" not in dbg:
                        ew0.tensor_tensor(zhT[:, :], zT[:, :], hT[:, :], ALU.mult)
                        ew0.tensor_scalar(omzT[:, :], zT[:, :], -1.0, 1.0, ALU.mult, ALU.add)
                    # V.h' is split: V.zh streams early (zh is ready at step
                    # start), only q = (1-z)*tanh(G1) stays on the chain, and
                    # h' = zh + q forms off-cycle (needed for hr + next zh).
                    pg1 = ps.tile([128, 512], F32, tag="pg1")
                    if not last:
                        pgr = ps.tile([128, 512], F32, tag="pgr")
                        pgz = ps.tile([128, 512], F32, tag="pgz")
                    if "no_mm" not in dbg:
                        emit_xproj(pg1, UT_h, 0, xs)
                        if not last:
                            emit_xproj(pgr, UrT, 2 * H, xs)
                            emit_xproj(pgz, UzT, H, xs)
                        emit_rec(pg1, WT_h, hrT)          # on-cycle (hr_{t-1})
                        if not last:
                            emit_rec(pgr, VrT, zhT, last=False)  # fill: tanh window
                            emit_rec(pgz, VzT, zhT, last=False)
                    if "no_act" not in dbg:
                        nc.scalar.activation(htT[:, :], pg1[:, 0:SW], AF.Tanh)
                    if "no_ew" not in dbg:
                        nc.vector.tensor_tensor(mT[:, :], omzT[:, :], htT[:, :], ALU.mult)
                        nc.vector.tensor_tensor(hT[:, :], zhT[:, :], mT[:, :], ALU.add)
                    if last:
                        return
                    if "no_mm" not in dbg:
                        emit_rec(pgr, VrT, mT)            # on-cycle (q)
                        emit_rec(pgz, VzT, mT)            # fills sigmoid window
                    if "no_act" not in dbg:
                        nc.scalar.activation(rT[:, :], pgr[:, 0:SW], AF.Sigmoid)
                    if "no_ew" not in dbg:
                        nc.vector.tensor_tensor(hrT[:, :], hT[:, :], rT[:, :], ALU.mult)
                    if "no_act" not in dbg:
                        nc.scalar.activation(zT[:, :], pgz[:, 0:SW], AF.Sigmoid)

                full_iters = (n_steps - 1) // unroll
                if full_iters > 1:
                    with tc.For_i(0, full_iters, 1,
                                  hint_engines=tuple(mybir.ALL_ENGINES)) as it:
                        for u in range(unroll):
                            step(it * unroll + u)
                    for t in range(full_iters * unroll, n_steps - 1):
                        step(t)
                else:
                    for t in range(n_steps - 1):
                        step(t)
                step(n_steps - 1, last=True)

                # output: y = h @ Wo.T (+ bo); hT is already in lhsT layout
                po = ps.tile([128, 512], F32, tag="po")
                for kt in range(KT):
                    nc.tensor.matmul(
                        po[0:B, 0:OUT], lhsT=hT[:, kt * B:(kt + 1) * B],
                        rhs=WoT[:, kt * OUT:(kt + 1) * OUT],
                        start=(kt == 0), stop=(kt == KT - 1 and not use_bias))
                if use_bias:
                    nc.tensor.matmul(
                        po[0:B, 0:OUT], lhsT=ones8[0:1, :],
                        rhs=bias_sb[0:1, 3 * H:3 * H + OUT],
                        start=False, stop=True)
                nc.vector.tensor_copy(ysb[0:B, :], po[0:B, 0:OUT])
                nc.sync.dma_start(Y_d[:, :], ysb[0:B, :])

    nc.compile()
    return nc


_CACHE = {}


def _get_nc(use_bias, n_steps=T, unroll=8):
    key = (use_bias, n_steps, unroll)
    if key not in _CACHE:
        _CACHE[key] = build(n_steps=n_steps, use_bias=use_bias, unroll=unroll)
    return _CACHE[key]


def _wt(W):
    # W [R, C] -> WT [128, (C//128) * R] fp16 with WT[p, kt*R + r] = W[r, kt*128 + p]
    R, C = W.shape
    return np.ascontiguousarray(
        W.T.reshape(C // 128, 128, R).transpose(1, 0, 2).reshape(128, -1),
        dtype=np.float16)


def prep_in_maps(inputs, n_cores=8):
    X = np.asarray(inputs["X"], dtype=np.float32)
    bt = X.shape[0] // n_cores
    use_bias = any(
        np.any(np.asarray(inputs[k]) != 0) for k in ("bx", "bz", "br", "bo") if k in inputs)

    weights = {
        "WhT": _wt(np.asarray(inputs["Wh"], np.float32)),
        "VzT": _wt(np.asarray(inputs["Vz"], np.float32)),
        "VrT": _wt(np.asarray(inputs["Vr"], np.float32)),
        "WxT": _wt(np.asarray(inputs["Wx"], np.float32)),
        "UzT": _wt(np.asarray(inputs["Uz"], np.float32)),
        "UrT": _wt(np.asarray(inputs["Ur"], np.float32)),
        "WoT": _wt(np.asarray(inputs["Wo"], np.float32)),
    }
    if use_bias:
        weights["biases"] = np.concatenate(
            [np.asarray(inputs[k], np.float32) for k in ("bx", "bz", "br", "bo")]
        ).reshape(1, -1).astype(np.float16)

    in_maps = []
    for c in range(n_cores):
        m = dict(weights)
        Xc = X[c * bt:(c + 1) * bt]  # [B, T, IN]
        # XT[p, t, ki*B + b] = X[b, t, ki*128 + p]
        m["XT"] = np.ascontiguousarray(
            Xc.reshape(bt, T, KI, 128).transpose(3, 1, 2, 0).reshape(128, T, KI * bt),
            dtype=np.float16)
        in_maps.append(m)
    return in_maps, use_bias


def kernel(**inputs):
    from concourse import bass_utils

    n_cores = 8
    in_maps, use_bias = prep_in_maps(inputs, n_cores)
    nc = _get_nc(use_bias)
    res = bass_utils.run_bass_kernel_spmd(nc, in_maps, core_ids=list(range(n_cores)))
    return np.concatenate([r["Y"] for r in res.results], axis=0)


if __name__ == "__main__":
    nc = build(n_steps=int(os.environ.get("STEPS", "16")), unroll=8)
    print("build OK")


# revision 28
# speedup vs baseline: 22.0193x; 1.0001x over previous
"""Trainium2 Bass kernel for a nonstandard GRU (gates computed after state update).

Strategy: data-parallel over batch (64 samples -> 8 cores x 8 samples).
Per core, the T=512 sequential recurrence runs entirely from SBUF with the
matmuls in weights-stationary orientation:
  - each gate matmul is out[128-chunk of H, B=8] = W_chunk^T.T @ h_chunk,
    i.e. lhsT = weight tile [K=128, M=128] (stationary), rhs = state
    [K=128, N=8] (moving) -> only 8 PE rows per matmul instruction.
  - gate outputs land in PSUM as [128, 8] tiles laid out side by side
    ([128, 64] per gate), which IS the transposed state layout the next
    matmul needs as rhs -> no PE transposes anywhere in the loop.
  - input projections (X @ Wx^T etc.) are folded in as 2 extra K-tiles
    from a pre-transposed XT; they are issued first so they fill PE gaps
    while the tanh/sigmoid/elementwise chain of the previous phase runs.
  - elementwise/activation ops see [128 partitions, 64 free] tensors.
"""

import os
import sys

sys.path.insert(0, "/opt/trn_rl_repo")

import numpy as np

import concourse.bass as bass
import concourse.mybir as mybir
import concourse.tile as tile
from concourse import bacc
from concourse.bass import ds

F32 = mybir.dt.float32
F16 = mybir.dt.float16  # matmul operands: 1 cycle/row (vs 4 for fp32), fp32 PSUM accum
AF = mybir.ActivationFunctionType
ALU = mybir.AluOpType

# problem dims (per core)
B = 8          # batch per core (64 / 8 cores)
T = 512
IN = 256
H = 1024
OUT = 256
KT = H // 128   # 8 k-tiles / out-tiles over hidden
KI = IN // 128  # 2 k-tiles over input
SW = KT * B     # 64: state width in transposed layout [128, SW]


def build(n_steps=T, use_bias=False, unroll=8, dbg=()):
    nc = bacc.Bacc("TRN2", target_bir_lowering=False)

    # All inputs are pre-transposed + fp16-converted on the HOST (see
    # _prep_weights/_prep_x below); device setup is then just straight DMAs.
    XT_d = nc.dram_tensor("XT", [128, T, KI * B], F16, kind="ExternalInput")
    WhT_d = nc.dram_tensor("WhT", [128, KT * H], F16, kind="ExternalInput")
    VzT_d = nc.dram_tensor("VzT", [128, KT * H], F16, kind="ExternalInput")
    VrT_d = nc.dram_tensor("VrT", [128, KT * H], F16, kind="ExternalInput")
    WxT_d = nc.dram_tensor("WxT", [128, KI * H], F16, kind="ExternalInput")
    UzT_d = nc.dram_tensor("UzT", [128, KI * H], F16, kind="ExternalInput")
    UrT_d = nc.dram_tensor("UrT", [128, KI * H], F16, kind="ExternalInput")
    WoT_d = nc.dram_tensor("WoT", [128, KT * OUT], F16, kind="ExternalInput")
    if use_bias:
        bias_d = nc.dram_tensor("biases", [1, 3 * H + OUT], F16, kind="ExternalInput")
    Y_d = nc.dram_tensor("Y", [B, OUT], F32, kind="ExternalOutput")

    with tile.TileContext(nc) as tc:
        with tc.tile_pool(name="state", bufs=1) as st:
            # persistent SBUF tensors
            # weight layouts: WT[p, kt*H + c] = W[c, kt*128 + p]
            #   -> lhsT(kt, mt) = WT[:, kt*H + mt*128 :][:128] is a [K=128, M=128]
            #      stationary tile of W^T
            WT_h = st.tile([128, KT * H], F16, tag="WT_h")
            VzT = st.tile([128, KT * H], F16, tag="VzT")
            VrT = st.tile([128, KT * H], F16, tag="VrT")
            UT_h = st.tile([128, KI * H], F16, tag="UT_h")
            UzT = st.tile([128, KI * H], F16, tag="UzT")
            UrT = st.tile([128, KI * H], F16, tag="UrT")
            WoT = st.tile([128, KT * OUT], F16, tag="WoT")
            XT = st.tile([128, T, KI * B], F16, tag="XT")
            ones8 = st.tile([1, B], F16, tag="ones8")
            bias_sb = st.tile([1, 3 * H + OUT], F16, tag="bias_sb")
            # transposed state [128, SW]: col ct*B + b <-> state[b, ct*128 + p]
            hT = st.tile([128, SW], F16, tag="hT")
            zT = st.tile([128, SW], F16, tag="zT")
            rT = st.tile([128, SW], F16, tag="rT")
            htT = st.tile([128, SW], F16, tag="htT")
            zhT = st.tile([128, SW], F16, tag="zhT")
            omzT = st.tile([128, SW], F16, tag="omzT")
            mT = st.tile([128, SW], F16, tag="mT")
            hrT = st.tile([128, SW], F16, tag="hrT")
            ysb = st.tile([128, OUT], F32, tag="ysb")

            nc.vector.memset(ones8[:], 1.0)
            for t_ in (hT, zT, rT, htT, zhT, omzT, mT, hrT):
                nc.vector.memset(t_[:], 0.0)
            if use_bias:
                nc.sync.dma_start(bias_sb[:, :], bias_d[:, :])
            else:
                nc.vector.memset(bias_sb[:], 0.0)

            # ---------- setup: straight DMAs of host-pre-transposed data ----
            # ordered by first use in the recurrence (WoT only needed at the
            # very end) so step 0 can start before the tail DMAs land
            nc.sync.dma_start(XT[:, 0:T // 8, :], XT_d[:, 0:T // 8, :])
            nc.sync.dma_start(UT_h[:, :], WxT_d[:, :])
            nc.sync.dma_start(UzT[:, :], UzT_d[:, :])
            nc.sync.dma_start(UrT[:, :], UrT_d[:, :])
            hw2 = KT * H // 2
            nc.sync.dma_start(WT_h[:, 0:hw2], WhT_d[:, 0:hw2])
            nc.sync.dma_start(WT_h[:, hw2:], WhT_d[:, hw2:])
            nc.sync.dma_start(VrT[:, 0:hw2], VrT_d[:, 0:hw2])
            nc.sync.dma_start(VrT[:, hw2:], VrT_d[:, hw2:])
            nc.sync.dma_start(VzT[:, 0:hw2], VzT_d[:, 0:hw2])
            nc.sync.dma_start(VzT[:, hw2:], VzT_d[:, hw2:])
            nc.sync.dma_start(XT[:, T // 8:T, :], XT_d[:, T // 8:T, :])
            nc.sync.dma_start(WoT[:, :], WoT_d[:, :])

            # ---------- recurrence ----------
            with tc.tile_pool(name="xp", bufs=3) as xp, \
                 tc.tile_pool(name="ps", bufs=1, space="PSUM") as ps:

                # PSUM start/stop semantics: start=True on the FIRST matmul
                # marks the whole 2KB zero region pending-zero; every later
                # matmul (start=False) zero-initializes the bytes it is
                # first to touch and accumulates thereafter. One group per
                # gate per bank-aligned psum tile. x-projection k-tiles are
                # issued first (they depend only on xst) so they fill PE gaps
                # while the previous phase's act/elementwise chain runs.
                def emit_xproj(pg, UT, boff, xs):
                    for mt in range(KT):
                        o = mt * B
                        for ki in range(KI):
                            nc.tensor.matmul(
                                pg[:, o:o + B],
                                lhsT=UT[:, ki * H + mt * 128:ki * H + mt * 128 + 128],
                                rhs=xs[ki],
                                start=(mt == 0 and ki == 0), stop=False)
                        if use_bias:
                            nc.tensor.matmul(
                                pg[:, o:o + B],
                                lhsT=bias_sb[0:1, boff + mt * 128:boff + (mt + 1) * 128],
                                rhs=ones8[0:1, :],
                                start=False, stop=False)

                def emit_rec(pg, WT, hsrc, last=True):
                    for kt in range(KT):
                        for mt in range(KT):
                            o = mt * B
                            nc.tensor.matmul(
                                pg[:, o:o + B],
                                lhsT=WT[:, kt * H + mt * 128:kt * H + mt * 128 + 128],
                                rhs=hsrc[:, kt * B:(kt + 1) * B],
                                start=False,
                                stop=(last and kt == KT - 1 and mt == KT - 1))

                def step(t_sc, last=False):
                    # last step: the r/z gates are dead (output needs only h),
                    # so skip their matmul streams, sigmoids, and hr
                    xst = xp.tile([128, 1, KI * B], F16, tag="xst")
                    # DVE beats gpsimd here: no Q7 launch cost, and the chain
                    # ops that consume zh/omz are on DVE anyway (no sem hop)
                    ew0 = nc.gpsimd if "use_gpsimd" in dbg else nc.vector
                    ew0.tensor_copy(xst[:], XT[:, ds(t_sc, 1), :])
                    xs = [xst[:, 0, ki * B:(ki + 1) * B] for ki in range(KI)]
                    # off critical path: zh = z*h, omz = 1-z (previous z,h)
                    if "no_ew" not in dbg:
                        ew0.tensor_tensor(zhT[:, :], zT[:, :], hT[:, :], ALU.mult)
                        ew0.tensor_scalar(omzT[:, :], zT[:, :], -1.0, 1.0, ALU.mult, ALU.add)
                    # V.h' is split: V.zh streams early (zh is ready at step
                    # start), only q = (1-z)*tanh(G1) stays on the chain, and
                    # h' = zh + q forms off-cycle (needed for hr + next zh).
                    pg1 = ps.tile([128, 512], F32, tag="pg1")
                    if not last:
                        pgr = ps.tile([128, 512], F32, tag="pgr")
                        pgz = ps.tile([128, 512], F32, tag="pgz")
                    if "no_mm" not in dbg:
                        emit_xproj(pg1, UT_h, 0, xs)
                        if not last:
                            emit_xproj(pgr, UrT, 2 * H, xs)
                            emit_xproj(pgz, UzT, H, xs)
                        emit_rec(pg1, WT_h, hrT)          # on-cycle (hr_{t-1})
                        if not last:
                            emit_rec(pgr, VrT, zhT, last=False)  # fill: tanh window
                            emit_rec(pgz, VzT, zhT, last=False)
                    if "no_act" not in dbg:
                        nc.scalar.activation(htT[:, :], pg1[:, 0:SW], AF.Tanh)
                    if "no_ew" not in dbg:
                        nc.vector.tensor_tensor(mT[:, :], omzT[:, :], htT[:, :], ALU.mult)
                        nc.vector.tensor_tensor(hT[:, :], zhT[:, :], mT[:, :], ALU.add)
                    if last:
                        return
                    if "no_mm" not in dbg:
                        emit_rec(pgr, VrT, mT)            # on-cycle (q)
                        emit_rec(pgz, VzT, mT)            # fills sigmoid window
                    if "no_act" not in dbg:
                        nc.scalar.activation(rT[:, :], pgr[:, 0:SW], AF.Sigmoid)
                    if "no_ew" not in dbg:
                        nc.vector.tensor_tensor(hrT[:, :], hT[:, :], rT[:, :], ALU.mult)
                    if "no_act" not in dbg:
                        nc.scalar.activation(zT[:, :], pgz[:, 0:SW], AF.Sigmoid)

                full_iters = (n_steps - 1) // unroll
                if full_iters > 1:
                    with tc.For_i(0, full_iters, 1,
                                  hint_engines=tuple(mybir.ALL_ENGINES)) as it:
                        for u in range(unroll):
                            step(it * unroll + u)
                    for t in range(full_iters * unroll, n_steps - 1):
                        step(t)
                else:
                    for t in range(n_steps - 1):
                        step(t)
                step(n_steps - 1, last=True)

                # output: y = h @ Wo.T (+ bo); hT is already in lhsT layout
                po = ps.tile([128, 512], F32, tag="po")
                for kt in range(KT):
                    nc.tensor.matmul(
                        po[0:B, 0:OUT], lhsT=hT[:, kt * B:(kt + 1) * B],
                        rhs=WoT[:, kt * OUT:(kt + 1) * OUT],
                        start=(kt == 0), stop=(kt == KT - 1 and not use_bias))
                if use_bias:
                    nc.tensor.matmul(
                        po[0:B, 0:OUT], lhsT=ones8[0:1, :],
                        rhs=bias_sb[0:1, 3 * H:3 * H + OUT],
                        start=False, stop=True)
                nc.vector.tensor_copy(ysb[0:B, :], po[0:B, 0:OUT])
                nc.sync.dma_start(Y_d[:, :], ysb[0:B, :])

    nc.compile()
    return nc


_CACHE = {}


def _get_nc(use_bias, n_steps=T, unroll=8):
    key = (use_bias, n_steps, unroll)
    if key not in _CACHE:
        _CACHE[key] = build(n_steps=n_steps, use_bias=use_bias, unroll=unroll)
    return _CACHE[key]


def _wt(W):
    # W [R, C] -> WT [128, (C//128) * R] fp16 with WT[p, kt*R + r] = W[r, kt*128 + p]
    R, C = W.shape
    return np.ascontiguousarray(
        W.T.reshape(C // 128, 128, R).transpose(1, 0, 2).reshape(128, -1),
        dtype=np.float16)


def prep_in_maps(inputs, n_cores=8):
    X = np.asarray(inputs["X"], dtype=np.float32)
    bt = X.shape[0] // n_cores
    use_bias = any(
        np.any(np.asarray(inputs[k]) != 0) for k in ("bx", "bz", "br", "bo") if k in inputs)

    weights = {
        "WhT": _wt(np.asarray(inputs["Wh"], np.float32)),
        "VzT": _wt(np.asarray(inputs["Vz"], np.float32)),
        "VrT": _wt(np.asarray(inputs["Vr"], np.float32)),
        "WxT": _wt(np.asarray(inputs["Wx"], np.float32)),
        "UzT": _wt(np.asarray(inputs["Uz"], np.float32)),
        "UrT": _wt(np.asarray(inputs["Ur"], np.float32)),
        "WoT": _wt(np.asarray(inputs["Wo"], np.float32)),
    }
    if use_bias:
        weights["biases"] = np.concatenate(
            [np.asarray(inputs[k], np.float32) for k in ("bx", "bz", "br", "bo")]
        ).reshape(1, -1).astype(np.float16)

    in_maps = []
    for c in range(n_cores):
        m = dict(weights)
        Xc = X[c * bt:(c + 1) * bt]  # [B, T, IN]
        # XT[p, t, ki*B + b] = X[b, t, ki*128 + p]
        m["XT"] = np.ascontiguousarray(
            Xc.reshape(bt, T, KI, 128).transpose(3, 1, 2, 0).reshape(128, T, KI * bt),
            dtype=np.float16)
        in_maps.append(m)
    return in_maps, use_bias


def kernel(**inputs):
    from concourse import bass_utils

    n_cores = 8
    in_maps, use_bias = prep_in_maps(inputs, n_cores)
    nc = _get_nc(use_bias)
    try:
        res = bass_utils.run_bass_kernel_spmd(nc, in_maps, core_ids=list(range(n_cores)))
    except Exception:
        # transient device errors (e.g. NRT_EXEC_UNIT_UNRECOVERABLE) usually
        # clear on a retry
        res = bass_utils.run_bass_kernel_spmd(nc, in_maps, core_ids=list(range(n_cores)))
    return np.concatenate([r["Y"] for r in res.results], axis=0)


if __name__ == "__main__":
    nc = build(n_steps=int(os.environ.get("STEPS", "16")), unroll=8)
    print("build OK")


# revision 30
# speedup vs baseline: 22.7506x; 1.0332x over previous
"""Trainium2 Bass kernel for a nonstandard GRU (gates computed after state update).

Strategy: data-parallel over batch (64 samples -> 8 cores x 8 samples).
All inputs are pre-transposed + fp16-converted on the host, so device setup
is 9 straight DMAs ordered by first use. Per core, the T=512 sequential
recurrence runs entirely from SBUF:
  - weights-stationary fp16 matmuls: lhsT = weight tile [K=128, M=128],
    rhs = state [K=128, N=8] -> out [128-chunk of H, 8] in fp32 PSUM.
    Gate outputs land as [128, 64] tiles that ARE the transposed state
    layout the next matmul consumes -> no PE transposes in the loop.
  - V.h' is split as V.zh + V.q (q = (1-z) * tanh(G1)): the V.zh streams
    and all x-projections depend only on early-available data and are
    issued as PE fill under the tanh/sigmoid latency windows; only q and
    hr = h'*r sit on the serial chain (DVE, fp16 2x mode).
  - per-step critical cycle ~2.48us: two PE->ACT->DVE->PE dependency loops
    (tanh, sigmoid-r), each paying PE drain 173ns + sem hops + ACT access
    latency; measured at the cost-model floor by perturbation probes.
  - the last step computes only the h-phase (r/z gates are dead code).
"""

import os
import sys

sys.path.insert(0, "/opt/trn_rl_repo")

import numpy as np

import concourse.bass as bass
import concourse.mybir as mybir
import concourse.tile as tile
from concourse import bacc
from concourse.bass import ds

F32 = mybir.dt.float32
F16 = mybir.dt.float16  # matmul operands: 1 cycle/row (vs 4 for fp32), fp32 PSUM accum
AF = mybir.ActivationFunctionType
ALU = mybir.AluOpType

# problem dims (per core)
B = 8          # batch per core (64 / 8 cores)
T = 512
IN = 256
H = 1024
OUT = 256
KT = H // 128   # 8 k-tiles / out-tiles over hidden
KI = IN // 128  # 2 k-tiles over input
SW = KT * B     # 64: state width in transposed layout [128, SW]


def build(n_steps=T, use_bias=False, unroll=8, dbg=()):
    nc = bacc.Bacc("TRN2", target_bir_lowering=False)

    # All inputs are pre-transposed + fp16-converted on the HOST (see
    # _prep_weights/_prep_x below); device setup is then just straight DMAs.
    XT_d = nc.dram_tensor("XT", [128, T, KI * B], F16, kind="ExternalInput")
    WhT_d = nc.dram_tensor("WhT", [128, KT * H], F16, kind="ExternalInput")
    VzT_d = nc.dram_tensor("VzT", [128, KT * H], F16, kind="ExternalInput")
    VrT_d = nc.dram_tensor("VrT", [128, KT * H], F16, kind="ExternalInput")
    WxT_d = nc.dram_tensor("WxT", [128, KI * H], F16, kind="ExternalInput")
    UzT_d = nc.dram_tensor("UzT", [128, KI * H], F16, kind="ExternalInput")
    UrT_d = nc.dram_tensor("UrT", [128, KI * H], F16, kind="ExternalInput")
    WoT_d = nc.dram_tensor("WoT", [128, KT * OUT], F16, kind="ExternalInput")
    if use_bias:
        bias_d = nc.dram_tensor("biases", [1, 3 * H + OUT], F16, kind="ExternalInput")
    Y_d = nc.dram_tensor("Y", [B, OUT], F32, kind="ExternalOutput")

    with tile.TileContext(nc) as tc:
        with tc.tile_pool(name="state", bufs=1) as st:
            # persistent SBUF tensors
            # weight layouts: WT[p, kt*H + c] = W[c, kt*128 + p]
            #   -> lhsT(kt, mt) = WT[:, kt*H + mt*128 :][:128] is a [K=128, M=128]
            #      stationary tile of W^T
            WT_h = st.tile([128, KT * H], F16, tag="WT_h")
            VzT = st.tile([128, KT * H], F16, tag="VzT")
            VrT = st.tile([128, KT * H], F16, tag="VrT")
            UT_h = st.tile([128, KI * H], F16, tag="UT_h")
            UzT = st.tile([128, KI * H], F16, tag="UzT")
            UrT = st.tile([128, KI * H], F16, tag="UrT")
            WoT = st.tile([128, KT * OUT], F16, tag="WoT")
            XT = st.tile([128, T, KI * B], F16, tag="XT")
            ones8 = st.tile([1, B], F16, tag="ones8")
            bias_sb = st.tile([1, 3 * H + OUT], F16, tag="bias_sb")
            # transposed state [128, SW]: col ct*B + b <-> state[b, ct*128 + p]
            hT = st.tile([128, SW], F16, tag="hT")
            zT = st.tile([128, SW], F16, tag="zT")
            rT = st.tile([128, SW], F16, tag="rT")
            htT = st.tile([128, SW], F16, tag="htT")
            zhT = st.tile([128, SW], F16, tag="zhT")
            omzT = st.tile([128, SW], F16, tag="omzT")
            mT = st.tile([128, SW], F16, tag="mT")
            hrT = st.tile([128, SW], F16, tag="hrT")
            ysb = st.tile([128, OUT], F32, tag="ysb")

            nc.vector.memset(ones8[:], 1.0)
            for t_ in (hT, zT, rT, htT, zhT, omzT, mT, hrT):
                nc.vector.memset(t_[:], 0.0)
            if use_bias:
                nc.sync.dma_start(bias_sb[:, :], bias_d[:, :])
            else:
                nc.vector.memset(bias_sb[:], 0.0)

            # ---------- setup: straight DMAs of host-pre-transposed data ----
            # ordered by first use in the recurrence (WoT only needed at the
            # very end) so step 0 can start before the tail DMAs land
            nc.sync.dma_start(XT[:, 0:T // 8, :], XT_d[:, 0:T // 8, :])
            nc.sync.dma_start(UT_h[:, :], WxT_d[:, :])
            nc.sync.dma_start(UzT[:, :], UzT_d[:, :])
            nc.sync.dma_start(UrT[:, :], UrT_d[:, :])
            hw2 = KT * H // 2
            nc.sync.dma_start(WT_h[:, 0:hw2], WhT_d[:, 0:hw2])
            nc.sync.dma_start(WT_h[:, hw2:], WhT_d[:, hw2:])
            nc.sync.dma_start(VrT[:, 0:hw2], VrT_d[:, 0:hw2])
            nc.sync.dma_start(VrT[:, hw2:], VrT_d[:, hw2:])
            nc.sync.dma_start(VzT[:, 0:hw2], VzT_d[:, 0:hw2])
            nc.sync.dma_start(VzT[:, hw2:], VzT_d[:, hw2:])
            nc.sync.dma_start(XT[:, T // 8:T, :], XT_d[:, T // 8:T, :])
            nc.sync.dma_start(WoT[:, :], WoT_d[:, :])

            # ---------- recurrence ----------
            with tc.tile_pool(name="xp", bufs=3) as xp, \
                 tc.tile_pool(name="ps", bufs=2, space="PSUM") as ps:

                # PSUM start/stop semantics: start=True on the FIRST matmul
                # marks the whole 2KB zero region pending-zero; every later
                # matmul (start=False) zero-initializes the bytes it is
                # first to touch and accumulates thereafter. One group per
                # gate per bank-aligned psum tile. x-projection k-tiles are
                # issued first (they depend only on xst) so they fill PE gaps
                # while the previous phase's act/elementwise chain runs.
                def emit_xproj(pg, UT, boff, xs):
                    for mt in range(KT):
                        o = mt * B
                        for ki in range(KI):
                            nc.tensor.matmul(
                                pg[:, o:o + B],
                                lhsT=UT[:, ki * H + mt * 128:ki * H + mt * 128 + 128],
                                rhs=xs[ki],
                                start=(mt == 0 and ki == 0), stop=False)
                        if use_bias:
                            nc.tensor.matmul(
                                pg[:, o:o + B],
                                lhsT=bias_sb[0:1, boff + mt * 128:boff + (mt + 1) * 128],
                                rhs=ones8[0:1, :],
                                start=False, stop=False)

                def emit_rec(pg, WT, hsrc, last=True):
                    for kt in range(KT):
                        for mt in range(KT):
                            o = mt * B
                            nc.tensor.matmul(
                                pg[:, o:o + B],
                                lhsT=WT[:, kt * H + mt * 128:kt * H + mt * 128 + 128],
                                rhs=hsrc[:, kt * B:(kt + 1) * B],
                                start=False,
                                stop=(last and kt == KT - 1 and mt == KT - 1))

                def step(t_sc, last=False):
                    # last step: the r/z gates are dead (output needs only h),
                    # so skip their matmul streams, sigmoids, and hr
                    xst = xp.tile([128, 1, KI * B], F16, tag="xst")
                    # DVE beats gpsimd here: no Q7 launch cost, and the chain
                    # ops that consume zh/omz are on DVE anyway (no sem hop)
                    ew0 = nc.gpsimd if "use_gpsimd" in dbg else nc.vector
                    ew0.tensor_copy(xst[:], XT[:, ds(t_sc, 1), :])
                    xs = [xst[:, 0, ki * B:(ki + 1) * B] for ki in range(KI)]
                    # off critical path: zh = z*h, omz = 1-z (previous z,h)
                    if "no_ew" not in dbg:
                        ew0.tensor_tensor(zhT[:, :], zT[:, :], hT[:, :], ALU.mult)
                        ew0.tensor_scalar(omzT[:, :], zT[:, :], -1.0, 1.0, ALU.mult, ALU.add)
                    # V.h' is split: V.zh streams early (zh is ready at step
                    # start), only q = (1-z)*tanh(G1) stays on the chain, and
                    # h' = zh + q forms off-cycle (needed for hr + next zh).
                    pg1 = ps.tile([128, 512], F32, tag="pg1")
                    if not last:
                        pgr = ps.tile([128, 512], F32, tag="pgr")
                        pgz = ps.tile([128, 512], F32, tag="pgz")
                    if "no_mm" not in dbg:
                        emit_xproj(pg1, UT_h, 0, xs)
                        if not last:
                            emit_xproj(pgr, UrT, 2 * H, xs)
                            emit_xproj(pgz, UzT, H, xs)
                        emit_rec(pg1, WT_h, hrT)          # on-cycle (hr_{t-1})
                        if not last:
                            emit_rec(pgr, VrT, zhT, last=False)  # fill: tanh window
                            emit_rec(pgz, VzT, zhT, last=False)
                    if "no_act" not in dbg:
                        nc.scalar.activation(htT[:, :], pg1[:, 0:SW], AF.Tanh)
                    if "no_ew" not in dbg:
                        nc.vector.tensor_tensor(mT[:, :], omzT[:, :], htT[:, :], ALU.mult)
                        nc.vector.tensor_tensor(hT[:, :], zhT[:, :], mT[:, :], ALU.add)
                    if last:
                        return
                    if "no_mm" not in dbg:
                        emit_rec(pgr, VrT, mT)            # on-cycle (q)
                        emit_rec(pgz, VzT, mT)            # fills sigmoid window
                    if "no_act" not in dbg:
                        nc.scalar.activation(rT[:, :], pgr[:, 0:SW], AF.Sigmoid)
                    if "no_ew" not in dbg:
                        nc.vector.tensor_tensor(hrT[:, :], hT[:, :], rT[:, :], ALU.mult)
                    if "no_act" not in dbg:
                        nc.scalar.activation(zT[:, :], pgz[:, 0:SW], AF.Sigmoid)

                full_iters = (n_steps - 1) // unroll
                if full_iters > 1:
                    with tc.For_i(0, full_iters, 1,
                                  hint_engines=tuple(mybir.ALL_ENGINES)) as it:
                        for u in range(unroll):
                            step(it * unroll + u)
                    for t in range(full_iters * unroll, n_steps - 1):
                        step(t)
                else:
                    for t in range(n_steps - 1):
                        step(t)
                step(n_steps - 1, last=True)

                # output: y = h @ Wo.T (+ bo); hT is already in lhsT layout
                po = ps.tile([128, 512], F32, tag="po")
                for kt in range(KT):
                    nc.tensor.matmul(
                        po[0:B, 0:OUT], lhsT=hT[:, kt * B:(kt + 1) * B],
                        rhs=WoT[:, kt * OUT:(kt + 1) * OUT],
                        start=(kt == 0), stop=(kt == KT - 1 and not use_bias))
                if use_bias:
                    nc.tensor.matmul(
                        po[0:B, 0:OUT], lhsT=ones8[0:1, :],
                        rhs=bias_sb[0:1, 3 * H:3 * H + OUT],
                        start=False, stop=True)
                nc.vector.tensor_copy(ysb[0:B, :], po[0:B, 0:OUT])
                nc.sync.dma_start(Y_d[:, :], ysb[0:B, :])

    nc.compile()
    return nc


_CACHE = {}


def _get_nc(use_bias, n_steps=T, unroll=8):
    key = (use_bias, n_steps, unroll)
    if key not in _CACHE:
        _CACHE[key] = build(n_steps=n_steps, use_bias=use_bias, unroll=unroll)
    return _CACHE[key]


def _wt(W):
    # W [R, C] -> WT [128, (C//128) * R] fp16 with WT[p, kt*R + r] = W[r, kt*128 + p]
    R, C = W.shape
    return np.ascontiguousarray(
        W.T.reshape(C // 128, 128, R).transpose(1, 0, 2).reshape(128, -1),
        dtype=np.float16)


def prep_in_maps(inputs, n_cores=8):
    X = np.asarray(inputs["X"], dtype=np.float32)
    bt = X.shape[0] // n_cores
    use_bias = any(
        np.any(np.asarray(inputs[k]) != 0) for k in ("bx", "bz", "br", "bo") if k in inputs)

    weights = {
        "WhT": _wt(np.asarray(inputs["Wh"], np.float32)),
        "VzT": _wt(np.asarray(inputs["Vz"], np.float32)),
        "VrT": _wt(np.asarray(inputs["Vr"], np.float32)),
        "WxT": _wt(np.asarray(inputs["Wx"], np.float32)),
        "UzT": _wt(np.asarray(inputs["Uz"], np.float32)),
        "UrT": _wt(np.asarray(inputs["Ur"], np.float32)),
        "WoT": _wt(np.asarray(inputs["Wo"], np.float32)),
    }
    if use_bias:
        weights["biases"] = np.concatenate(
            [np.asarray(inputs[k], np.float32) for k in ("bx", "bz", "br", "bo")]
        ).reshape(1, -1).astype(np.float16)

    in_maps = []
    for c in range(n_cores):
        m = dict(weights)
        Xc = X[c * bt:(c + 1) * bt]  # [B, T, IN]
        # XT[p, t, ki*B + b] = X[b, t, ki*128 + p]
        m["XT"] = np.ascontiguousarray(
            Xc.reshape(bt, T, KI, 128).transpose(3, 1, 2, 0).reshape(128, T, KI * bt),
            dtype=np.float16)
        in_maps.append(m)
    return in_maps, use_bias


def kernel(**inputs):
    from concourse import bass_utils

    n_cores = 8
    in_maps, use_bias = prep_in_maps(inputs, n_cores)
    nc = _get_nc(use_bias)
    try:
        res = bass_utils.run_bass_kernel_spmd(nc, in_maps, core_ids=list(range(n_cores)))
    except Exception:
        # transient device errors (e.g. NRT_EXEC_UNIT_UNRECOVERABLE) usually
        # clear on a retry
        res = bass_utils.run_bass_kernel_spmd(nc, in_maps, core_ids=list(range(n_cores)))
    return np.concatenate([r["Y"] for r in res.results], axis=0)


if __name__ == "__main__":
    nc = build(n_steps=int(os.environ.get("STEPS", "16")), unroll=8)
    print("build OK")
